# revision 10
# baseline (speedup 1.0000x reference)
"""TT-adapter linear kernel for TRN2, data-parallel over batch on 8 NeuronCores.

Math: out = x @ W.T + b + ALPHA * TT(x).  TT is linear in x, so it folds into
the weight on the host:  Wc = W + ALPHA * T,  out = x @ Wc.T + b.

Per core: one batch element, [2048,1024] @ [1024,1024] in bf16 (f32 PSUM),
~54.6us pure-TensorE floor at 2.4GHz.

Schedule (v2):
  - x is chunked [NS=4 sc][DO=8 d][128][512] so the d-staircase is fed at
    384KiB/round (w_d 256K + x(0,d) 128K) -- under the 1.76us/round compute.
  - Phase A (sc=0): d-outer staircase across 8 PSUM banks (o=0..7), matmuls
    start as (w_d, x(0,d)) pairs land.
  - Phases sc=1..3: o-outer / d-inner, so groups finish staggered and
    evictions pipeline; bank o gated on the previous sc's eviction.
  - Evictions on ACT add bias and convert to bf16 (halves output DMA bytes);
    host converts back to f32.
  - Input DMA issue (~0.6us each on the issuing engine) is split: Sync does
    w/x(sc0)/x(sc1)/bias then all output DMAs; Scalar does x(sc2)/x(sc3)
    before its eviction loop.
  - The first (w0, x(0,0)) DMA issues are hoisted into the framework entry
    block (before the engine drain/barrier) so their data transfers overlap
    the ~7us engine-init preamble; only 2 are hoisted because issue time
    before the barrier delays every engine's start.
  - Warm-up matmuls ramp the PE clock (HAM) during the remaining input
    latency window.
"""

import numpy as np
import ml_dtypes
from contextlib import ExitStack

import concourse.bass as bass  # noqa: F401
import concourse.mybir as mybir
from concourse import bacc
from concourse.bass_utils import run_bass_kernel_spmd

ALPHA = 16.0
B, S, D = 8, 2048, 1024
P = 128
DO = D // P          # 8 contraction tiles
OO = D // P          # 8 output tiles
SCH = 512
NS = S // SCH        # 4 s-chunks
NG = OO * NS         # 32 groups
NBANK = 8
NSLOT = 16
NWARM = 6

_NC = None


def _build_nc():
    nc = bacc.Bacc("TRN2", target_bir_lowering=False, debug=False)
    xt = nc.declare_dram_parameter("xt", [NS, DO, P, SCH], mybir.dt.bfloat16, isOutput=False)
    wt = nc.declare_dram_parameter("wt", [DO, P, D], mybir.dt.bfloat16, isOutput=False)
    bi = nc.declare_dram_parameter("bi", [P, OO], mybir.dt.float32, isOutput=False)
    out = nc.declare_dram_parameter("out", [OO, P, S], mybir.dt.bfloat16, isOutput=True)

    hoist = []  # DMA issues moved into the entry block pre-barrier

    with ExitStack() as ctx:
        block = ctx.enter_context(nc.Block(no_gpsimd_drain=True))
        # one sem per (w_d, x(0,d)) pair: HWDGE completions are unordered
        s_p = [ctx.enter_context(nc.semaphore(f"s_p{d}")) for d in range(DO)]
        # per-(sc,d) sems for sc=1..3
        s_x = {sc: [ctx.enter_context(nc.semaphore(f"s_x{sc}_{d}")) for d in range(DO)]
               for sc in (1, 2, 3)}
        s_bias = ctx.enter_context(nc.semaphore("s_bias"))
        s_mm = ctx.enter_context(nc.semaphore("s_mm"))
        s_ev = ctx.enter_context(nc.semaphore("s_ev"))
        s_slot = [ctx.enter_context(nc.semaphore(f"s_slot{k}")) for k in range(NSLOT)]
        bias_sb = ctx.enter_context(nc.sbuf_tensor("bias_sb", [P, OO], mybir.dt.float32))
        w_sb = ctx.enter_context(nc.sbuf_tensor("w_sb", [P, DO, D], mybir.dt.bfloat16))
        x_sb = ctx.enter_context(nc.sbuf_tensor("x_sb", [P, NS, DO, SCH], mybir.dt.bfloat16))
        ot_sb = ctx.enter_context(nc.sbuf_tensor("ot_sb", [P, NSLOT, SCH], mybir.dt.bfloat16))
        ps = [ctx.enter_context(nc.psum_tensor(f"ps{b}", [P, SCH], mybir.dt.float32))
              for b in range(NBANK)]

        @block.sync
        def _(sync: bass.BassEngine):
            # first pair: recorded for hoisting into the entry block
            hoist.append(sync.dma_start(out=w_sb[:, 0, :], in_=wt[0]).then_inc(s_p[0], 16))
            hoist.append(sync.dma_start(out=x_sb[:, 0, 0, :], in_=xt[0, 0]).then_inc(s_p[0], 16))
            for d in range(1, DO):
                sync.dma_start(out=w_sb[:, d, :], in_=wt[d]).then_inc(s_p[d], 16)
                sync.dma_start(out=x_sb[:, 0, d, :], in_=xt[0, d]).then_inc(s_p[d], 16)
            sync.dma_start(out=bias_sb[:, :], in_=bi[:, :]).then_inc(s_bias, 16)
            for d in range(DO):
                sync.dma_start(out=x_sb[:, 1, d, :], in_=xt[1, d]).then_inc(s_x[1][d], 16)
            for k in range(NSLOT):
                sync.wait_ge(s_slot[k], 16 * (NG // NSLOT))

        @block.tensor
        def _(tensor: bass.BassEngine):
            # HAM warm-up: ramp the PE clock while the first tiles land
            for _ in range(NWARM):
                tensor.matmul(
                    ps[0][:, 0:256], w_sb[:, 0, 0:P], x_sb[:, 0, 0, 0:256],
                    start=True, stop=True,
                )
            # phase A: sc=0, d-outer staircase over banks o=0..7
            for d in range(DO):
                tensor.wait_ge(s_p[d], 32)
                for o in range(OO):
                    mmi = tensor.matmul(
                        ps[o][:, :],
                        w_sb[:, d, o * P:(o + 1) * P],
                        x_sb[:, 0, d, :],
                        start=(d == 0),
                        stop=(d == DO - 1),
                    )
                    if d == DO - 1:
                        mmi.then_inc(s_mm, 1)
            # phases sc=1..3: o-outer, d-inner (staggered finishes)
            for sc in (1, 2, 3):
                for o in range(OO):
                    g = OO * sc + o
                    tensor.wait_ge(s_ev, g - NBANK + 1)
                    for d in range(DO):
                        if o == 0:
                            tensor.wait_ge(s_x[sc][d], 16)
                        mmi = tensor.matmul(
                            ps[o][:, :],
                            w_sb[:, d, o * P:(o + 1) * P],
                            x_sb[:, sc, d, :],
                            start=(d == 0),
                            stop=(d == DO - 1),
                        )
                        if d == DO - 1:
                            mmi.then_inc(s_mm, 1)

        @block.scalar
        def _(scalar: bass.BassEngine):
            # late x chunks on the ACT HWDGE ring, then all output DMAs,
            # each right behind its eviction (gated on s_ev)
            for sc in (2, 3):
                for d in range(DO):
                    scalar.dma_start(out=x_sb[:, sc, d, :], in_=xt[sc, d]).then_inc(
                        s_x[sc][d], 16)
            for g in range(NG):
                sc, o = g // OO, g % OO
                scalar.wait_ge(s_ev, g + 1)
                scalar.dma_start(
                    out=out[o, :, sc * SCH:(sc + 1) * SCH],
                    in_=ot_sb[:, g % NSLOT, :],
                ).then_inc(s_slot[g % NSLOT], 16)

        @block.vector
        def _(vector: bass.BassEngine):
            # evictions on DVE: PSUM f32 + per-partition bias -> SBUF bf16
            vector.wait_ge(s_bias, 16)
            for g in range(NG):
                sc, o = g // OO, g % OO
                vector.wait_ge(s_mm, g + 1)
                if g >= NSLOT:
                    vector.wait_ge(s_slot[g % NSLOT], 16 * (g // NSLOT))
                vector.tensor_scalar_add(
                    ot_sb[:, g % NSLOT, :], ps[o][:, :], bias_sb[:, o:o + 1]
                ).then_inc(s_ev, 1)

    # hoist the first (w0, x00) DMA issues to the very top of the entry block
    # (ahead of the per-engine register setup) so their transfers overlap the
    # engine-init preamble
    mf = nc.main_func
    entry = mf.blocks[0]
    moved = []
    want = 2
    for blk in mf.blocks[1:]:
        for inst in list(blk.instructions):
            if isinstance(inst, mybir.InstDMACopy) and len(moved) < want:
                blk.instructions.remove(inst)
                moved.append(inst)
        if len(moved) >= want:
            break
    # position 1: right after the entry InstCall marker
    for j, inst in enumerate(moved):
        entry.instructions.insert(1 + j, inst)

    nc.compile()
    return nc


def _get_nc():
    global _NC
    if _NC is None:
        _NC = _build_nc()
    return _NC


def _merged_weight_T(W, b, core0, core1, core2, core3, core4, core5):
    f8 = np.float64
    A = core0[0].astype(f8)
    Bm = np.einsum('ap,pbq->abq', A, core1.astype(f8))
    C = np.einsum('abq,qcr->abcr', Bm, core2.astype(f8))
    Phi = C.transpose(2, 1, 0, 3).reshape(D, 8)
    Dn = np.einsum('paq,qbr->pabr', core3.astype(f8), core4.astype(f8))
    E = np.einsum('pabq,qc->pabc', Dn, core5[:, :, 0].astype(f8))
    Psi = E.reshape(8, D)
    WcT = W.T.astype(f8) + ALPHA * (Phi @ Psi)
    return WcT.astype(np.float32)


def _prep_in_maps(x, W, b, core0, core1, core2, core3, core4, core5):
    WcT = _merged_weight_T(W, b, core0, core1, core2, core3, core4, core5)
    wt = WcT.reshape(DO, P, D).astype(ml_dtypes.bfloat16)
    bi = np.ascontiguousarray(b.reshape(OO, P).T).astype(np.float32)
    in_maps = []
    for bb in range(B):
        # xt[sc, d, p, j] = x[b, 512*sc + j, 128*d + p]
        xt = np.ascontiguousarray(
            x[bb].reshape(NS, SCH, DO, P).transpose(0, 2, 3, 1)
        ).astype(ml_dtypes.bfloat16)
        in_maps.append({"xt": xt, "wt": wt, "bi": bi})
    return in_maps


def _gather(results):
    outs = []
    for bb in range(B):
        o = np.asarray(results[bb]["out"]).astype(np.float32)
        outs.append(o.transpose(2, 0, 1).reshape(S, D))
    return np.ascontiguousarray(np.stack(outs))


def run(inputs, **spmd_kwargs):
    inputs = {k: np.asarray(v) for k, v in inputs.items()}
    in_maps = _prep_in_maps(**inputs)
    nc = _get_nc()
    res = run_bass_kernel_spmd(nc, in_maps, core_ids=list(range(B)), **spmd_kwargs)
    return _gather(res.results), res


def kernel(x, W, b, core0, core1, core2, core3, core4, core5):
    out, _ = run(dict(x=x, W=W, b=b, core0=core0, core1=core1, core2=core2,
                      core3=core3, core4=core4, core5=core5))
    return out


# revision 16
# speedup vs baseline: 1.0314x; 1.0314x over previous
"""TT-adapter linear kernel for TRN2, data-parallel over batch on 8 NeuronCores.

Math: out = x @ W.T + b + ALPHA * TT(x).  TT is linear in x, so it folds into
the weight on the host:  Wc = W + ALPHA * T,  out = x @ Wc.T + b.

Per core: one batch element, [2048,1024] @ [1024,1024] in bf16 (f32 PSUM),
~54.6us pure-TensorE floor at 2.4GHz.

Schedule (v2):
  - x is chunked [NS=4 sc][DO=8 d][128][512] so the d-staircase is fed at
    384KiB/round (w_d 256K + x(0,d) 128K) -- under the 1.76us/round compute.
  - Phase A (sc=0): d-outer staircase across 8 PSUM banks (o=0..7), matmuls
    start as (w_d, x(0,d)) pairs land.
  - Phases sc=1..3: o-outer / d-inner, so groups finish staggered and
    evictions pipeline; bank o gated on the previous sc's eviction.
  - Evictions on ACT add bias and convert to bf16 (halves output DMA bytes);
    host converts back to f32.
  - Input DMA issue (~0.6us each on the issuing engine) is split: Sync does
    w/x(sc0)/x(sc1)/bias then all output DMAs; Scalar does x(sc2)/x(sc3)
    before its eviction loop.
  - The first (w0, x(0,0)) DMA issues are hoisted into the framework entry
    block (before the engine drain/barrier) so their data transfers overlap
    the ~7us engine-init preamble; only 2 are hoisted because issue time
    before the barrier delays every engine's start.
  - Warm-up matmuls ramp the PE clock (HAM) during the remaining input
    latency window.
"""

import numpy as np
import ml_dtypes
from contextlib import ExitStack

import concourse.bass as bass  # noqa: F401
import concourse.mybir as mybir
from concourse import bacc
from concourse.bass_utils import run_bass_kernel_spmd

ALPHA = 16.0
B, S, D = 8, 2048, 1024
P = 128
DO = D // P          # 8 contraction tiles
OO = D // P          # 8 output tiles
SCH = 512
NS = S // SCH        # 4 s-chunks
NG = OO * NS         # 32 groups
NBANK = 8
NSLOT = 16
NWARM = 3

_NC = None


def _build_nc():
    nc = bacc.Bacc("TRN2", target_bir_lowering=False, debug=False)
    xt = nc.declare_dram_parameter("xt", [NS, DO, P, SCH], mybir.dt.bfloat16, isOutput=False)
    wt = nc.declare_dram_parameter("wt", [DO, P, D], mybir.dt.bfloat16, isOutput=False)
    bi = nc.declare_dram_parameter("bi", [P, OO], mybir.dt.float32, isOutput=False)
    out = nc.declare_dram_parameter("out", [OO, P, S], mybir.dt.bfloat16, isOutput=True)

    hoist = []  # DMA issues moved into the entry block pre-barrier

    with ExitStack() as ctx:
        block = ctx.enter_context(nc.Block(no_gpsimd_drain=True))
        # one sem per (w_d, x(0,d)) pair: HWDGE completions are unordered
        s_p = [ctx.enter_context(nc.semaphore(f"s_p{d}")) for d in range(DO)]
        # per-(sc,d) sems for sc=1..3
        s_x = {sc: [ctx.enter_context(nc.semaphore(f"s_x{sc}_{d}")) for d in range(DO)]
               for sc in (1, 2, 3)}
        s_bias = ctx.enter_context(nc.semaphore("s_bias"))
        s_mm = ctx.enter_context(nc.semaphore("s_mm"))
        s_ev = ctx.enter_context(nc.semaphore("s_ev"))
        s_slot = [ctx.enter_context(nc.semaphore(f"s_slot{k}")) for k in range(NSLOT)]
        bias_sb = ctx.enter_context(nc.sbuf_tensor("bias_sb", [P, OO], mybir.dt.float32))
        w_sb = ctx.enter_context(nc.sbuf_tensor("w_sb", [P, DO, D], mybir.dt.bfloat16))
        x_sb = ctx.enter_context(nc.sbuf_tensor("x_sb", [P, NS, DO, SCH], mybir.dt.bfloat16))
        ot_sb = ctx.enter_context(nc.sbuf_tensor("ot_sb", [P, NSLOT, SCH], mybir.dt.bfloat16))
        ps = [ctx.enter_context(nc.psum_tensor(f"ps{b}", [P, SCH], mybir.dt.float32))
              for b in range(NBANK)]

        @block.sync
        def _(sync: bass.BassEngine):
            # first pair recorded for hoisting into the entry block (issues
            # during the init preamble); later x(0,d) go on the Scalar ring
            # so phase-A rounds are fed by two parallel HWDGE rings
            hoist.append(sync.dma_start(out=w_sb[:, 0, :], in_=wt[0]).then_inc(s_p[0], 16))
            hoist.append(sync.dma_start(out=x_sb[:, 0, 0, :], in_=xt[0, 0]).then_inc(s_p[0], 16))
            for d in range(1, DO):
                sync.dma_start(out=w_sb[:, d, :], in_=wt[d]).then_inc(s_p[d], 16)
            sync.dma_start(out=bias_sb[:, :], in_=bi[:, :]).then_inc(s_bias, 16)
            for d in range(DO):
                sync.dma_start(out=x_sb[:, 1, d, :], in_=xt[1, d]).then_inc(s_x[1][d], 16)
            for k in range(NSLOT):
                sync.wait_ge(s_slot[k], 16 * (NG // NSLOT))

        @block.tensor
        def _(tensor: bass.BassEngine):
            # HAM warm-up: ramp the PE clock while the first tiles land
            for _ in range(NWARM):
                tensor.matmul(
                    ps[0][:, 0:256], w_sb[:, 0, 0:P], x_sb[:, 0, 0, 0:256],
                    start=True, stop=True,
                )
            # phase A: sc=0, d-outer staircase over banks o=0..7
            for d in range(DO):
                tensor.wait_ge(s_p[d], 32)
                for o in range(OO):
                    mmi = tensor.matmul(
                        ps[o][:, :],
                        w_sb[:, d, o * P:(o + 1) * P],
                        x_sb[:, 0, d, :],
                        start=(d == 0),
                        stop=(d == DO - 1),
                    )
                    if d == DO - 1:
                        mmi.then_inc(s_mm, 1)
            # phases sc=1..3: o-outer, d-inner (staggered finishes)
            for sc in (1, 2, 3):
                for o in range(OO):
                    g = OO * sc + o
                    tensor.wait_ge(s_ev, g - NBANK + 1)
                    for d in range(DO):
                        if o == 0:
                            tensor.wait_ge(s_x[sc][d], 16)
                        mmi = tensor.matmul(
                            ps[o][:, :],
                            w_sb[:, d, o * P:(o + 1) * P],
                            x_sb[:, sc, d, :],
                            start=(d == 0),
                            stop=(d == DO - 1),
                        )
                        if d == DO - 1:
                            mmi.then_inc(s_mm, 1)

        @block.scalar
        def _(scalar: bass.BassEngine):
            # x chunks on the ACT HWDGE ring, then all output DMAs behind
            # their evictions (s_ev)
            for d in range(1, DO):
                scalar.dma_start(out=x_sb[:, 0, d, :], in_=xt[0, d]).then_inc(s_p[d], 16)
            for sc in (2, 3):
                for d in range(DO):
                    scalar.dma_start(out=x_sb[:, sc, d, :], in_=xt[sc, d]).then_inc(
                        s_x[sc][d], 16)
            for g in range(NG):
                sc, o = g // OO, g % OO
                scalar.wait_ge(s_ev, g + 1)
                scalar.dma_start(
                    out=out[o, :, sc * SCH:(sc + 1) * SCH],
                    in_=ot_sb[:, g % NSLOT, :],
                ).then_inc(s_slot[g % NSLOT], 16)

        @block.vector
        def _(vector: bass.BassEngine):
            # evictions on DVE: PSUM f32 + per-partition bias -> SBUF bf16
            vector.wait_ge(s_bias, 16)
            for g in range(NG):
                sc, o = g // OO, g % OO
                vector.wait_ge(s_mm, g + 1)
                if g >= NSLOT:
                    vector.wait_ge(s_slot[g % NSLOT], 16 * (g // NSLOT))
                vector.tensor_scalar_add(
                    ot_sb[:, g % NSLOT, :], ps[o][:, :], bias_sb[:, o:o + 1]
                ).then_inc(s_ev, 1)

    # hoist the recorded (w0 on Sync, x00 on Scalar) DMA issues into the
    # entry block so their transfers overlap the engine-init preamble
    mf = nc.main_func
    entry = mf.blocks[0]
    targets = [bi.ins for bi in hoist]
    for blk in mf.blocks[1:]:
        for inst in list(blk.instructions):
            if inst in targets:
                blk.instructions.remove(inst)
    for j, inst in enumerate(targets):
        entry.instructions.insert(1 + j, inst)

    nc.compile()
    return nc


def _get_nc():
    global _NC
    if _NC is None:
        _NC = _build_nc()
    return _NC


def _merged_weight_T(W, b, core0, core1, core2, core3, core4, core5):
    f8 = np.float64
    A = core0[0].astype(f8)
    Bm = np.einsum('ap,pbq->abq', A, core1.astype(f8))
    C = np.einsum('abq,qcr->abcr', Bm, core2.astype(f8))
    Phi = C.transpose(2, 1, 0, 3).reshape(D, 8)
    Dn = np.einsum('paq,qbr->pabr', core3.astype(f8), core4.astype(f8))
    E = np.einsum('pabq,qc->pabc', Dn, core5[:, :, 0].astype(f8))
    Psi = E.reshape(8, D)
    WcT = W.T.astype(f8) + ALPHA * (Phi @ Psi)
    return WcT.astype(np.float32)


def _prep_in_maps(x, W, b, core0, core1, core2, core3, core4, core5):
    WcT = _merged_weight_T(W, b, core0, core1, core2, core3, core4, core5)
    wt = WcT.reshape(DO, P, D).astype(ml_dtypes.bfloat16)
    bi = np.ascontiguousarray(b.reshape(OO, P).T).astype(np.float32)
    in_maps = []
    for bb in range(B):
        # xt[sc, d, p, j] = x[b, 512*sc + j, 128*d + p]
        xt = np.ascontiguousarray(
            x[bb].reshape(NS, SCH, DO, P).transpose(0, 2, 3, 1)
        ).astype(ml_dtypes.bfloat16)
        in_maps.append({"xt": xt, "wt": wt, "bi": bi})
    return in_maps


def _gather(results):
    outs = []
    for bb in range(B):
        o = np.asarray(results[bb]["out"]).astype(np.float32)
        outs.append(o.transpose(2, 0, 1).reshape(S, D))
    return np.ascontiguousarray(np.stack(outs))


def run(inputs, **spmd_kwargs):
    inputs = {k: np.asarray(v) for k, v in inputs.items()}
    in_maps = _prep_in_maps(**inputs)
    nc = _get_nc()
    res = run_bass_kernel_spmd(nc, in_maps, core_ids=list(range(B)), **spmd_kwargs)
    return _gather(res.results), res


def kernel(x, W, b, core0, core1, core2, core3, core4, core5):
    out, _ = run(dict(x=x, W=W, b=b, core0=core0, core1=core1, core2=core2,
                      core3=core3, core4=core4, core5=core5))
    return out
